# revision 40
# baseline (speedup 1.0000x reference)
"""Trainium2 Bass kernel: dwconv-QKV attention block, data-parallel over batch.

fp8e4m3 DoubleRow matmuls for scores (zero-padded second k-tile) and AV
(real g-block pairs) quarter the PE time of attention; the Activation
engine runs exps only (all drains/copies moved to DVE/Pool), making exp
throughput the roofline. Conv stays hybrid: 4 taps as PE diag matmuls,
5 taps as DVE 4x tensor-scalar ops with adds split DVE(2x)/Pool.

Layout notes:
- qT/kT [128, 1552] fp8: cols 0..1023 = tokens 1..1024, col 1024 = cls,
  cols 1040..1552 = zeros (DoubleRow zero-tile; ldweights pair stride must
  be a multiple of 16 bytes, offsets even).
- vpp[dgi] [128, 2*912] fp8: two 12-head V strips (76 = 64 v-cols + 12
  one-hot denominator cols) for key blocks g=2dgi, 2dgi+1.
- E pair tiles [128, 2048] fp8: exp of block g lands in half g%2.
"""
import sys

sys.path.insert(0, "/opt/trn_rl_repo")

import numpy as np
import ml_dtypes

from concourse import bass, bacc, mybir, tile
from concourse.bass_utils import run_bass_kernel_spmd
from concourse.masks import make_identity

F32 = mybir.dt.float32
BF16 = mybir.dt.bfloat16
F8 = mybir.dt.float8e4
DR = mybir.MatmulPerfMode.DoubleRow
NCORES = 8
B, T, C, HEADS = 16, 1025, 768, 12
BL = B // NCORES
EPS = 1e-5
SCALE = float(C) ** -0.5
SW = 76  # per-head strip in vpp/ops: 64 v-cols + 12 one-hot denominator cols
VW = 12 * SW  # 912, one g-block's strip row; multiple of 16 (ldweights rule)
ZQ = 1040  # zero-region offset in qT/kT (multiple of 16)
QW = 1552  # qT/kT width: 1040 data+pad, 512 zeros
CHUNKS = [(0, 510), (510, 1020), (1020, 1088)]  # conv col chunks, 34-aligned
PE_TAPS = (4, 5, 6, 7, 8)  # conv taps done as PE diag matmuls
NPE = len(PE_TAPS)


def _dr3(sl, stride, n):
    """3-dim AP for DoubleRow: insert a [stride, 2] pair dim into a 2D slice."""
    return bass.AP(tensor=sl.tensor, offset=sl.offset,
                   ap=[list(sl.ap[0]), [stride, 2], [1, n]])


def _build(nc):
    x = nc.declare_dram_parameter("x", [BL, T, C], F32, isOutput=False)
    diagc = nc.declare_dram_parameter("diagc", [3, 6, 128, 9, 128], BF16, isOutput=False)
    biasc = nc.declare_dram_parameter("biasc", [128, 18], F32, isOutput=False)
    pwc = nc.declare_dram_parameter("pwc", [128, 6 * C], BF16, isOutput=False)
    selc = nc.declare_dram_parameter("selc", [12, C], BF16, isOutput=False)
    oneh = nc.declare_dram_parameter("oneh", [128, 144], BF16, isOutput=False)
    wcol = nc.declare_dram_parameter("wcol", [128, 162], F32, isOutput=False)
    out = nc.declare_dram_parameter("out", [BL, T, C], F32, isOutput=True)

    from contextlib import ExitStack
    with nc.allow_low_precision(reason="fp8/bf16 compute, rel-err budget 2e-2"), \
         tile.TileContext(nc, trace_sim=False) as tc, ExitStack() as stk:
        sing = stk.enter_context(tc.tile_pool(name="sing", bufs=1))
        ld = stk.enter_context(tc.tile_pool(name="ld", bufs=2))
        eap = stk.enter_context(tc.tile_pool(name="eap", bufs=3))
        scp = stk.enter_context(tc.tile_pool(name="scp", bufs=2, space="PSUM"))
        avp = stk.enter_context(tc.tile_pool(name="avp", bufs=1, space="PSUM"))
        cvp = stk.enter_context(tc.tile_pool(name="cvp", bufs=1, space="PSUM"))

        ident = sing.tile([128, 128], BF16, tag="ident")
        make_identity(nc, ident[:])
        ident8 = sing.tile([128, 128], F8, tag="ident8")
        make_identity(nc, ident8[:])
        bsb = sing.tile([128, 18], F32, tag="bsb")
        pw2s = sing.tile([128, 6 * C], BF16, tag="pw2s")
        sel2s = sing.tile([12, C], BF16, tag="sel2s")
        onehs = sing.tile([128, 144], BF16, tag="onehs")
        wcols = sing.tile([128, 162], F32, tag="wcols")

        def emit_param_dmas_early():
            nc.sync.dma_start(onehs[:], oneh[:])
            nc.sync.dma_start(bsb[:], biasc[:])
            nc.sync.dma_start(wcols[:], wcol[:])

        def emit_param_dmas_late():
            nc.sync.dma_start(pw2s[:], pwc[:])
            nc.sync.dma_start(sel2s[:], selc[:])

        qT = [[sing.tile([128, QW], F8, tag=f"qT{b}_{cc}", name=f"qT{b}_{cc}")
               for cc in range(6)] for b in range(BL)]
        kT = [[sing.tile([128, QW], F8, tag=f"kT{b}_{cc}", name=f"kT{b}_{cc}")
               for cc in range(6)] for b in range(BL)]
        # vppb[b]: 8 key-block strips; block g at cols g*VW:(g+1)*VW
        vppb = [sing.tile([128, 8 * VW], F8, tag=f"vppb{b}", name=f"vppb{b}")
                for b in range(BL)]
        # vp0[b]: cls-key strip (cols 0:VW) + zeros (VW:2VW) for DR lhsT
        vp0 = [sing.tile([128, 2 * VW], F8, tag=f"vp0{b}", name=f"vp0{b}")
               for b in range(BL)]
        oT2 = [[sing.tile([128, T], BF16, tag=f"oT2{b}_{hh}", name=f"oT2{b}_{hh}")
                for hh in range(6)] for b in range(BL)]
        xT = [[sing.tile([128, 1160], BF16, tag=f"xT{b}_{cc}", name=f"xT{b}_{cc}")
               for cc in range(6)] for b in range(BL)]
        dd = [sing.tile([76, T], BF16, tag=f"dd{b}", name=f"dd{b}") for b in range(BL)]
        ddq = [sing.tile([76, 12], F32, tag=f"ddq{b}", name=f"ddq{b}") for b in range(BL)]
        ddr = sing.tile([12, T], F32, tag="ddr")
        rr = sing.tile([12, T], F32, tag="rr")
        rrb = sing.tile([12, T], BF16, tag="rrb")
        clsb = [sing.tile([1, C], BF16, tag=f"clsb{b}", name=f"clsb{b}") for b in range(BL)]

        # alternating PSUM conv/scratch tiles (1 bank each)
        cv_i = [0]

        def cvt(shape, dtype, nm):
            cv_i[0] ^= 1
            return cvp.tile(shape, dtype, tag=f"cv{cv_i[0]}", name=nm)

        # ---------- input staging ----------
        def emit_x_dmas(b):
            clsf = ld.tile([1, C], F32, tag="clsf", bufs=1)
            nc.gpsimd.dma_start(clsf[:], x[b, 0:1, :])
            nc.vector.tensor_copy(clsb[b][:], clsf[:])
            clscf = ld.tile([128, 6], F32, tag="clscf", bufs=1)
            nc.gpsimd.dma_start(clscf[:], x[b, 0, :].rearrange("(cc p) -> p cc", p=128))
            clsc8 = ld.tile([128, 6], F8, tag="clsc8", bufs=1)
            nc.vector.tensor_copy(clsc8[:], clscf[:])
            for cc in range(6):
                nc.vector.tensor_copy(qT[b][cc][:, 1024:1025], clsc8[:, cc:cc + 1])
                nc.vector.tensor_copy(kT[b][cc][:, 1024:1025], clsc8[:, cc:cc + 1])
            # vp0: zeros everywhere, then cls v-values + one-hot cols
            nc.gpsimd.memset(vp0[b][:], 0.0)
            for h in range(12):
                nc.gpsimd.tensor_copy(
                    vp0[b][0:1, h * SW:h * SW + 64], clsb[b][:, h * 64:(h + 1) * 64])
                # mirror at partition 32 so head-B cls AV matches E2's base
                nc.gpsimd.tensor_copy(
                    vp0[b][32:33, h * SW:h * SW + 64], clsb[b][:, h * 64:(h + 1) * 64])
            dst0 = vp0[b][:, 0:VW].rearrange("p (h s) -> p h s", s=SW)[:, :, 64:SW]
            nc.gpsimd.tensor_copy(dst0, onehs[:].rearrange("p (h s) -> p h s", s=12))
            for g in range(8):
                dst = vppb[b][:, g * VW:(g + 1) * VW].rearrange(
                    "p (h s) -> p h s", s=SW)[:, :, 64:SW]
                nc.gpsimd.tensor_copy(dst, onehs[:].rearrange("p (h s) -> p h s", s=12))
            nc.gpsimd.memset(dd[b][64:76, :], 0.0)

        def xT_init(b, eng):
            # only the conv halo cells need zeroing; strips overwrite the rest
            for cc in range(6):
                t = xT[b][cc]
                eng.memset(t[:, 0:34], 0.0)        # top halo row
                eng.memset(t[:, 1122:1156], 0.0)   # bottom halo row
                eng.memset(t[:, 33:33 + 34 * 33].rearrange(
                    "p (r w) -> p r w", w=34)[:, :, 0:2], 0.0)  # side halos
                eng.memset(t[:, 1156:1160], 0.0)

        def emit_qk_zeros(b, cc):
            # zero region for DoubleRow zero-tile trick
            nc.gpsimd.memset(qT[b][cc][:, ZQ:QW], 0.0)
            nc.gpsimd.memset(kT[b][cc][:, ZQ:QW], 0.0)

        def xT_chunk_thunks(b, split_cc0=False):
            """thunks: per tt-pair [2x dma+cast, 6x (2 bf16 transposes + 1 copy)].
            split_cc0: all loads + cc0 strips in `first`, cc1-5 strips in `rest`
            so conv-cc0 can start as soon as possible."""
            first, rest = [], []
            for tp in range(4):
                holder = {}
                for ti in range(2):
                    def t_load(b=b, tp=tp, ti=ti, holder=holder):
                        tt = 2 * tp + ti
                        xn = ld.tile([128, C], F32, tag="xn", bufs=3)
                        eng_dma = nc.sync if tt % 2 == 0 else nc.gpsimd
                        eng_dma.dma_start(xn[:], x[b, 1 + tt * 128:1 + (tt + 1) * 128, :])
                        xb = ld.tile([128, C], BF16,
                                     tag="xb8" if split_cc0 else "xb",
                                     bufs=8 if split_cc0 else 3)
                        # Act is idle during startup: batch-0 casts go there
                        if split_cc0:
                            nc.scalar.copy(xb[:], xn[:])
                        else:
                            nc.gpsimd.tensor_copy(xb[:], xn[:])
                        holder[ti] = xb
                    first.append(t_load)
                for cc in range(6):
                    def t_tr(b=b, tp=tp, cc=cc, holder=holder):
                        pt = cvt([128, 256], BF16, nm="ptx")
                        for ti in range(2):
                            nc.tensor.transpose(
                                pt[:, ti * 128:(ti + 1) * 128],
                                holder[ti][:, cc * 128:(cc + 1) * 128], ident[:])
                        dst = xT[b][cc][:, 0:1156].rearrange(
                            "p (r w) -> p r w", w=34)[:, 1 + tp * 8:1 + tp * 8 + 8, 1:33]
                        nc.vector.tensor_copy(
                            dst.rearrange("p (t r) w -> p t r w", t=2),
                            pt[:].rearrange("p (t r w) -> p t r w", t=2, w=32))
                    (first if cc == 0 or not split_cc0 else rest).append(t_tr)
            if split_cc0:
                return first, rest
            return first + rest

        # ---------- conv thunks for one (b, cc) ----------
        def conv_thunks(b, cc, pe_taps=False, qs=(0, 1, 2)):
            thunks = []
            holder = {}
            for q in qs:
                def t_dgq(b=b, cc=cc, q=q, holder=holder, pe_taps=pe_taps):
                    if pe_taps:
                        dg = ld.tile([128, 1152], BF16, tag="dg9", bufs=3)
                        nc.sync.dma_start(
                            dg[:].rearrange("p (a c) -> p a c", a=9), diagc[q, cc])
                    else:
                        dg = ld.tile([128, NPE * 128], BF16, tag="dg")
                        nc.sync.dma_start(
                            dg[:].rearrange("p (a c) -> p a c", a=NPE),
                            diagc[q, cc, :, PE_TAPS[0]:PE_TAPS[0] + NPE, :])
                    holder[q] = dg
                thunks.append(t_dgq)

                def t_vtaps(b=b, cc=cc, q=q, holder=holder):
                    if pe_taps:
                        return
                    # taps 0..4 off PE: tap0 on DVE ts (scale+bias fused); taps
                    # 1-4 as DVE 4x-mode scalar-mults + adds split DVE/Pool.
                    wb = (q * 6 + cc) * 9
                    y = ld.tile([128, 1088], BF16, tag="y", name="y")
                    holder[(q, "y")] = y
                    nc.vector.tensor_scalar(
                        y[:], xT[b][cc][:, 0:1088], wcols[:, wb:wb + 1],
                        bsb[:, q * 6 + cc:q * 6 + cc + 1],
                        mybir.AluOpType.mult, mybir.AluOpType.add)
                    for t9 in range(1, PE_TAPS[0]):
                        dy, dx = t9 // 3, t9 % 3
                        off = 35 + (dy - 1) * 34 + (dx - 1)
                        tmp = ld.tile([128, 1088], BF16, tag="tmp", name="tmp")
                        eng_m = nc.vector if t9 < 3 else nc.gpsimd
                        eng_m.tensor_scalar_mul(
                            tmp[:], xT[b][cc][:, off:1088 + off],
                            wcols[:, wb + t9:wb + t9 + 1])
                        nc.gpsimd.tensor_add(y[:], y[:], tmp[:])
                thunks.append(t_vtaps)
                if q == 2:
                    def t_vt(holder=holder):
                        holder["vt"] = ld.tile([128, 1024], F8, tag="vt", name="vt", bufs=1)
                    thunks.append(t_vt)
                for ci, (o0, o1) in enumerate(CHUNKS):
                    def t_chunk(b=b, cc=cc, q=q, ci=ci, o0=o0, o1=o1, holder=holder,
                                pe_taps=pe_taps):
                        dg = holder[q]
                        osz = o1 - o0
                        nb = osz // 34
                        ct = cvt([128, 512], F32, nm="ct")
                        taps = range(9) if pe_taps else PE_TAPS
                        for ti, t9 in enumerate(taps):
                            dy, dx = t9 // 3, t9 % 3
                            off = 35 + (dy - 1) * 34 + (dx - 1) + o0
                            nc.tensor.matmul(
                                ct[:, 0:osz], dg[:, ti * 128:(ti + 1) * 128],
                                xT[b][cc][:, off:off + osz],
                                start=(ti == 0), stop=(t9 == 8))
                        src = ct[:, 0:osz].rearrange("p (r w) -> p r w", w=34)[:, :, 0:32]
                        col0 = (o0 // 34) * 32
                        if q < 2:
                            dstT = (qT if q == 0 else kT)[b][cc]
                            dst = dstT[:, col0:col0 + nb * 32].rearrange(
                                "p (r w) -> p r w", w=32)
                        else:
                            vt = holder["vt"]
                            dst = vt[:, col0:col0 + nb * 32].rearrange(
                                "p (r w) -> p r w", w=32)
                        if pe_taps:
                            nc.vector.tensor_scalar_add(
                                dst, src, bsb[:, q * 6 + cc:q * 6 + cc + 1])
                        else:
                            y = holder[(q, "y")]
                            yv = y[:, o0:o1].rearrange("p (r w) -> p r w", w=34)[:, :, 0:32]
                            nc.vector.scalar_tensor_tensor(
                                dst, src, 1.0, yv,
                                mybir.AluOpType.mult, mybir.AluOpType.add)
                    thunks.append(t_chunk)
            for g4 in (range(2) if 2 in qs else []):
                def t_vtr(b=b, cc=cc, g4=g4, holder=holder):
                    vt = holder["vt"]
                    # fp8 transpose mode requires output element step of 2
                    pt = cvt([128, 1024], F8, nm="ptv")
                    for ti in range(4):
                        g2 = 4 * g4 + ti
                        sl = pt[:, ti * 256:ti * 256 + 256]
                        o2 = bass.AP(tensor=sl.tensor, offset=sl.offset,
                                     ap=[list(sl.ap[0]), [2, 128]])
                        nc.tensor.transpose(o2, vt[:, g2 * 128:(g2 + 1) * 128],
                                            ident8[:])
                    # four key blocks' strips for this cc's head pair
                    sl = vppb[b][0:128, 4 * g4 * VW + 2 * cc * SW:]
                    dst = bass.AP(tensor=sl.tensor, offset=sl.offset,
                                  ap=[list(sl.ap[0]), [VW, 4], [SW, 2], [1, 64]])
                    sp = pt[:, 0:1024]
                    src = bass.AP(tensor=sp.tensor, offset=sp.offset,
                                  ap=[list(sp.ap[0]), [256, 4], [128, 2], [2, 64]])
                    nc.vector.tensor_copy(dst, src)
                thunks.append(t_vtr)
            return thunks

        # ---------- feeder ----------
        feeder = []
        feeder2 = []

        def drain(k):
            popped = 0
            while popped < k and feeder:
                feeder.pop(0)()
                popped += 1
            if popped < k and feeder2:
                feeder2.pop(0)()  # at most one chunky woven item per call

        def drain_all():
            drain(len(feeder))
            while feeder2:
                feeder2.pop(0)()

        # ---------- attention stage for (b, hh) ----------
        def stage_head(b, hh):
            """cls-key scores + exp; emitted as soon as q/k conv for this cc
            is drained so Act never waits at stage boundaries."""
            cc = hh
            # cls-key (col 1024) scores for BOTH heads: zero-padded block kz
            # lhsT puts head A's row at partition 0, head B's at 32.
            kz = eap.tile([128, 96], F8, tag="kz", name="kz", bufs=2)
            nc.vector.memset(kz[:], 0.0)
            nc.vector.tensor_copy(kz[0:64, 0:1], kT[b][cc][0:64, 1024:1025])
            nc.vector.tensor_copy(kz[64:128, 32:33], kT[b][cc][64:128, 1024:1025])
            scz = scp.tile([128, 1024], F32, tag="sc", name="scz")
            for (l0, l1) in [(0, 512), (512, 1024)]:
                nc.tensor.matmul(scz[0:33, l0:l1],
                                 _dr3(kz[0:128, 0:33], 48, 33),
                                 _dr3(qT[b][cc][0:128, l0:l1], ZQ - l0, 512),
                                 start=True, stop=True, perf_mode=DR)
            E2 = eap.tile([33, 1024], F8, tag="E2", name="E2", bufs=2)
            nc.scalar.activation(E2[:], scz[0:33, :],
                                 mybir.ActivationFunctionType.Exp, scale=SCALE)
            return E2

        def attn_body(b, hh, E2, per_slot, w2=0):
            cc = hh
            for hl in range(2):
                h = 2 * hh + hl
                r0 = hl * 64
                ops = avp.tile([SW, 1024], F32, tag="av", name="ops")
                # cls-key rank-1 AV starts the accumulation (DR: lhsT pair-tile
                # is the vp0 zeros half, rhs pair-tile is stride-0 aliased)
                for (l0, l1) in [(0, 512), (512, 1024)]:
                    nc.tensor.matmul(
                        ops[:, l0:l1],
                        _dr3(vp0[b][hl * 32:hl * 32 + 1, h * SW:(h + 1) * SW], VW, SW),
                        _dr3(E2[hl * 32:hl * 32 + 1, l0:l1], 0, 512),
                        start=True, stop=False, perf_mode=DR)

                def emit_scores(g):
                    c0 = g * 128
                    sc = scp.tile([128, 1024], F32, tag="sc", name="sc")
                    for (l0, l1) in [(0, 512), (512, 1024)]:
                        nc.tensor.matmul(
                            sc[0:128, l0:l1],
                            _dr3(kT[b][cc][r0:r0 + 64, c0:c0 + 128], ZQ - c0, 128),
                            _dr3(qT[b][cc][r0:r0 + 64, l0:l1], ZQ - l0, 512),
                            start=True, stop=True, perf_mode=DR)
                    return sc

                # keep scores 2 blocks ahead of the exps so Act never waits
                scs = {0: emit_scores(0), 1: emit_scores(1)}
                Eps = {}
                for g in range(8):
                    dgi, par = g // 2, g % 2
                    if par == 0:
                        Eps[dgi] = eap.tile([128, 2048], F8, tag="E", name="E", bufs=4)
                    nc.scalar.activation(
                        Eps[dgi][:, par * 1024:(par + 1) * 1024], scs.pop(g)[:],
                        mybir.ActivationFunctionType.Exp, scale=SCALE)
                    if par == 1:
                        for (l0, l1) in [(0, 512), (512, 1024)]:
                            nc.tensor.matmul(
                                ops[:, l0:l1],
                                _dr3(vppb[b][0:128, 2 * dgi * VW + h * SW:
                                             2 * dgi * VW + (h + 1) * SW], VW, SW),
                                _dr3(Eps[dgi][0:128, l0:l0 + 512], 1024, 512),
                                start=False, stop=(dgi == 3), perf_mode=DR)
                    drain(per_slot)
                    # woven batch-0 norm/proj: mid-slot, never at stage heads
                    if w2 and g in (2, 5) and feeder2:
                        feeder2.pop(0)()
                    if g < 6:
                        scs[g + 2] = emit_scores(g + 2)
                # drain ops for head h
                nc.vector.tensor_copy(oT2[b][hh][r0:r0 + 64, 0:1024], ops[0:64, :])
                nc.vector.tensor_add(dd[b][64:76, 0:1024], dd[b][64:76, 0:1024],
                                     ops[64:SW, 0:1024])
                drain(3)
            # cls-query (col 1024) tail for this pair: short-lived PSUM scratch
            def cls_tail(b=b, hh=hh, cc=cc):
              sctl = cvt([128, 18], F32, nm="sctl")
              nc.vector.memset(sctl[:], 0.0)
              for hl in range(2):
                r0 = hl * 64
                for g in range(9):
                    if g == 0:
                        tsz, c0 = 1, 1024
                    else:
                        tsz, c0 = 128, (g - 1) * 128
                    nc.tensor.matmul(
                        sctl[0:tsz, hl * 9 + g:hl * 9 + g + 1],
                        kT[b][cc][r0:r0 + 64, c0:c0 + tsz],
                        qT[b][cc][r0:r0 + 64, 1024:1025], start=True, stop=True,
                        skip_group_check=True)
              E9 = eap.tile([128, 36], F8, tag="E9", name="E9")
              nc.scalar.activation(
                  E9[:].rearrange("p (n o) -> p n o", o=2)[:, :, 0:1],
                  sctl[:].rearrange("p (n o) -> p n o", o=1),
                  mybir.ActivationFunctionType.Exp, scale=SCALE)
              drain(2)
              otl = cvt([SW, 2], F32, nm="otl")
              for hl in range(2):
                h = 2 * hh + hl
                for g in range(9):
                    col = 2 * (hl * 9 + g)
                    if g == 0:
                        lhsT = vp0[b][0:1, h * SW:(h + 1) * SW]
                        tsz = 1
                    else:
                        lhsT = vppb[b][0:128, (g - 1) * VW + h * SW:
                                      (g - 1) * VW + (h + 1) * SW]
                        tsz = 128
                    nc.tensor.matmul(
                        otl[:, hl:hl + 1], lhsT,
                        E9[0:tsz, col:col + 1],
                        start=(g == 0), stop=(g == 8), skip_group_check=True)
              nc.vector.tensor_copy(oT2[b][hh][0:64, 1024:1025], otl[0:64, 0:1])
              nc.vector.tensor_copy(oT2[b][hh][64:128, 1024:1025], otl[0:64, 1:2])
              nc.vector.tensor_copy(ddq[b][64:76, 2 * hh:2 * hh + 2], otl[64:SW, 0:2])
            cls_tail()

        # ---------- normalize + proj for batch b ----------
        def norm_b(b, nrows):
            # engine partition bases must be 32-aligned: always start at 64/0
            nc.vector.tensor_reduce(
                dd[b][64:64 + nrows, 1024:1025], ddq[b][64:64 + nrows, :],
                mybir.AxisListType.X, mybir.AluOpType.add)
            # custom-DVE recip breaks at partition base 64 on HW: stage to base 0
            nc.gpsimd.tensor_copy(ddr[0:nrows, :], dd[b][64:64 + nrows, :])
            nc.vector.reciprocal_approx_fast(rr[0:nrows, :], ddr[0:nrows, :])
            nc.gpsimd.tensor_copy(rrb[0:nrows, :], rr[0:nrows, :])

        def norm_hh(b, hh, woven=True):
            # broadcast head reciprocals to 128 partitions via sel matmul, then
            # multiply oT2 straight from PSUM (no staging copy). Woven calls
            # use the 1-bank cvt tiles to keep the scores PSUM tag free.
            if woven:
                rp1 = cvt([128, 512], F32, nm="rp1")
                nc.tensor.matmul(rp1[:, 0:512], sel2s[0:12, hh * 128:(hh + 1) * 128],
                                 rrb[0:12, 0:512], start=True, stop=True)
                nc.vector.tensor_mul(oT2[b][hh][:, 0:512], oT2[b][hh][:, 0:512],
                                     rp1[:])
                rp2 = cvt([128, 512], F32, nm="rp2")
                nc.tensor.matmul(rp2[:, 0:512], sel2s[0:12, hh * 128:(hh + 1) * 128],
                                 rrb[0:12, 512:1024], start=True, stop=True)
                nc.vector.tensor_mul(oT2[b][hh][:, 512:1024],
                                     oT2[b][hh][:, 512:1024], rp2[:, 0:512])
            else:
                rp = scp.tile([128, 1024], F32, tag="sc", name="rp")
                for (l0, l1) in [(0, 512), (512, 1024)]:
                    nc.tensor.matmul(rp[:, l0:l1],
                                     sel2s[0:12, hh * 128:(hh + 1) * 128],
                                     rrb[0:12, l0:l1], start=True, stop=True)
                nc.vector.tensor_mul(oT2[b][hh][:, 0:1024], oT2[b][hh][:, 0:1024],
                                     rp[:])
            rp3 = cvt([128, 16], F32, nm="rp3")
            nc.tensor.matmul(rp3[:, 0:1], sel2s[0:12, hh * 128:(hh + 1) * 128],
                             rrb[0:12, 1024:1025], start=True, stop=True)
            nc.vector.tensor_mul(oT2[b][hh][:, 1024:1025], oT2[b][hh][:, 1024:1025],
                                 rp3[:, 0:1])

        def proj_lt(b, lt, woven=True):
            # oT2 col j = token j+1 for j<1024; col 1024 = token 0 (cls)
            lsz = 128 if lt < 8 else 1
            t0 = 1 + lt * 128 if lt < 8 else 0
            ob = eap.tile([128, C], F32, tag="ob", name="ob", bufs=2)
            if woven:
                for (e0, e1) in [(0, 512), (512, 768)]:
                    pp = cvt([128, 512], F32, nm="pp")
                    for hh2 in range(6):
                        nc.tensor.matmul(
                            pp[0:lsz, 0:e1 - e0],
                            oT2[b][hh2][:, lt * 128:lt * 128 + lsz],
                            pw2s[:, hh2 * C + e0:hh2 * C + e1],
                            start=(hh2 == 0), stop=(hh2 == 5))
                    nc.vector.tensor_copy(ob[0:lsz, e0:e1], pp[0:lsz, 0:e1 - e0])
            else:
                pp = scp.tile([128, C], F32, tag="sc", name="pp")
                for hh2 in range(6):
                    for (e0, e1) in [(0, 512), (512, 768)]:
                        nc.tensor.matmul(
                            pp[0:lsz, e0:e1],
                            oT2[b][hh2][:, lt * 128:lt * 128 + lsz],
                            pw2s[:, hh2 * C + e0:hh2 * C + e1],
                            start=(hh2 == 0), stop=(hh2 == 5))
                nc.vector.tensor_copy(ob[0:lsz, :], pp[0:lsz, :])
            nc.sync.dma_start(out[b, t0:t0 + lsz, :], ob[0:lsz, :])

        # ================= emission schedule =================
        xT_init(0, nc.vector)
        xt0_first, xt0_rest = xT_chunk_thunks(0, split_cc0=True)
        for t in xt0_first[:4]:
            t()
        emit_param_dmas_early()
        for t in xt0_first[4:]:
            t()
        emit_x_dmas(0)
        emit_qk_zeros(0, 0)
        # stage 0: conv q/k inline, head, then v inline
        for t in conv_thunks(0, 0, pe_taps=True, qs=(1, 0)):
            t()
        E2_head = [stage_head(0, 0)]
        for t in conv_thunks(0, 0, pe_taps=True, qs=(2,)):
            t()
        feeder.extend(xt0_rest)
        qk_end = [0]
        all_end = [0]
        for ccx in range(1, 6):
            feeder.append(lambda b=0, ccx=ccx: emit_qk_zeros(b, ccx))
            feeder.extend(conv_thunks(0, ccx, qs=(1, 0)))
            qk_end.append(len(feeder))
            feeder.extend(conv_thunks(0, ccx, qs=(2,)))
            all_end.append(len(feeder))
        feeder.append(lambda: emit_param_dmas_late())
        feeder.append(lambda: emit_x_dmas(1))
        feeder.append(lambda: xT_init(1, nc.gpsimd))
        feeder.extend(xT_chunk_thunks(1))
        for ccx in range(6):
            feeder.append(lambda b=1, ccx=ccx: emit_qk_zeros(b, ccx))
            feeder.extend(conv_thunks(1, ccx, qs=(1, 0)))
            qk_end.append(len(feeder))
            feeder.extend(conv_thunks(1, ccx, qs=(2,)))
            all_end.append(len(feeder))
        n_total = len(feeder)

        def drain_to(target):
            k = target - (n_total - len(feeder))
            if k > 0:
                drain(k)

        stages = [(0, hh) for hh in range(6)] + [(1, hh) for hh in range(6)]
        for s, (b, hh) in enumerate(stages):
            if s > 0:
                drain_to(qk_end[s])
                E2_head[0] = stage_head(b, hh)
                drain_to(all_end[s])

            if s == 11:
                # heads 0..9 of batch 1 are final (their stages + cls tails done)
                feeder2.append(lambda: norm_b(1, 10))
                feeder2.extend(
                    [lambda hh1=hh1: norm_hh(1, hh1) for hh1 in range(5)])
            nxt = qk_end[s + 1] if s + 1 < 12 else n_total
            need = nxt - (n_total - len(feeder))
            per_slot = max(1, -(-need // 18)) if (need > 0 or feeder2) else 1
            attn_body(b, hh, E2_head[0], per_slot, w2=(6 <= s))
            if s == 5:
                norm_b(0, 12)
                for hh0 in range(6):
                    feeder2.append(lambda hh0=hh0: norm_hh(0, hh0))
                for lt in range(9):
                    feeder2.append(lambda lt=lt: proj_lt(0, lt))
            if s == 11:
                # last stage's norm inside the stage: kernel tail is proj only
                norm_b(1, 12)
                norm_hh(1, 5)
        drain_all()
        # tail: batch-1 projection
        for lt in range(9):
            proj_lt(1, lt, woven=False)
    return nc


_CACHE = {}


def _get_nc():
    if "nc" not in _CACHE:
        nc = bacc.Bacc("TRN2", target_bir_lowering=False, debug=False,
                       enable_asserts=False, num_devices=NCORES)
        _build(nc)
        nc.compile()
        _CACHE["nc"] = nc
    return _CACHE["nc"]


def _prep_weights(w, g, bb, m, v):
    s = (np.asarray(g) / np.sqrt(np.asarray(v) + EPS)).astype(np.float32)
    w9 = np.asarray(w).reshape(C, 9).astype(np.float32) * s[:, None]
    bias = (np.asarray(bb) - np.asarray(m) * s).astype(np.float32)
    return w9, bias


def _make_in_maps(inputs):
    x = np.asarray(inputs["x"], dtype=np.float32)
    diagc = np.zeros((3, 6, 128, 9, 128), dtype=ml_dtypes.bfloat16)
    wcolv = np.zeros((128, 162), dtype=np.float32)
    biasc = np.zeros((128, 18), dtype=np.float32)
    idx = np.arange(128)
    for q, pre in enumerate(["q", "k", "v"]):
        w9, bias = _prep_weights(
            inputs[f"w_{pre}"], inputs[f"bn_{pre}_g"], inputs[f"bn_{pre}_b"],
            inputs[f"bn_{pre}_m"], inputs[f"bn_{pre}_v"])
        for cc in range(6):
            for t in range(9):
                diagc[q, cc, idx, t, idx] = w9[cc * 128:(cc + 1) * 128, t].astype(
                    ml_dtypes.bfloat16)
                wcolv[:, (q * 6 + cc) * 9 + t] = w9[cc * 128:(cc + 1) * 128, t]
            biasc[:, q * 6 + cc] = bias[cc * 128:(cc + 1) * 128]
    # pwc[p, hh*C+e] = proj_w.T[hh*128+p, e]
    pwT = np.ascontiguousarray(np.asarray(inputs["proj_w"], np.float32).T)
    pwc = np.ascontiguousarray(
        pwT.reshape(6, 128, C).transpose(1, 0, 2).reshape(128, 6 * C)).astype(
        ml_dtypes.bfloat16)
    # selc[j, hh*128+m] = 1 if (m<64 and j==2hh) or (m>=64 and j==2hh+1)
    oneh = np.tile(np.eye(12).reshape(1, 144), (128, 1)).astype(ml_dtypes.bfloat16)
    selc = np.zeros((12, C), dtype=ml_dtypes.bfloat16)
    for hh in range(6):
        selc[2 * hh, hh * 128:hh * 128 + 64] = 1
        selc[2 * hh + 1, hh * 128 + 64:(hh + 1) * 128] = 1
    in_maps = []
    for ci in range(NCORES):
        in_maps.append({
            "x": np.ascontiguousarray(x[ci * BL:(ci + 1) * BL]),
            "diagc": diagc, "biasc": biasc, "pwc": pwc, "selc": selc,
            "oneh": np.ascontiguousarray(oneh), "wcol": wcolv,
        })
    return in_maps


def kernel(x, w_q, bn_q_g, bn_q_b, bn_q_m, bn_q_v,
           w_k, bn_k_g, bn_k_b, bn_k_m, bn_k_v,
           w_v, bn_v_g, bn_v_b, bn_v_m, bn_v_v,
           proj_w, proj_b, h, w, **_):
    inputs = dict(x=x, w_q=w_q, bn_q_g=bn_q_g, bn_q_b=bn_q_b, bn_q_m=bn_q_m,
                  bn_q_v=bn_q_v, w_k=w_k, bn_k_g=bn_k_g, bn_k_b=bn_k_b,
                  bn_k_m=bn_k_m, bn_k_v=bn_k_v, w_v=w_v, bn_v_g=bn_v_g,
                  bn_v_b=bn_v_b, bn_v_m=bn_v_m, bn_v_v=bn_v_v, proj_w=proj_w)
    nc = _get_nc()
    in_maps = _make_in_maps(inputs)
    res = run_bass_kernel_spmd(nc, in_maps, core_ids=list(range(NCORES)))
    outs = [res.results[ci]["out"] for ci in range(NCORES)]
    full = np.concatenate(outs, axis=0).astype(np.float32)
    full += np.asarray(proj_b, np.float32)[None, None, :]
    return full
